# revision 10
# baseline (speedup 1.0000x reference)
"""Trainium2 Bass kernel for nn_CatEmbedder (gnn_message_passing).

Strategy (v2):
- Batch-parallel across 8 cores (4096 samples each), table replicated.
- Gather via dma_gather (one SWDGE instruction per range-pass per block)
  instead of 1600 per-field indirect DMAs (which serialized ~1.8ms on Q7).
  dma_gather needs int16 indices, so the 100k-row table is split into 4
  quarters of 25001 rows (each with a trailing zero row for padding) and
  each block issues 4 gathers. Per-sample quarter-counts vary, so samples
  are globally sorted by count profile and dealt round-robin to cores;
  each block pads every sample to the block's per-quarter max with
  zero-row lookups. Padded slots contribute relu(t_s) to the global
  branch, which is subtracted exactly via a per-sample pad-count
  correction term.
- fp16 data path everywhere (4x faster matmuls than fp32, half the
  gather bytes); fp32 PSUM accumulation. rel err ~4e-4 vs 2e-2 budget.
- Algebraic fold: gacc_f = (e_f + summed/PROBE) @ (ga_W*PROBE/CD) + ga_b,
  so the per-field bias becomes one broadcast add on the embeddings.
"""

import sys
import types
import inspect

import numpy as np

sys.path.insert(0, "/opt/trn_rl_repo")

# ---- problem constants ----
B, F, D, NCT = 32768, 50, 64, 100000
PROBE, ALPHA = 39.0, 0.5
NF = F + 1
CD = NF + PROBE
NCORES = 8
BS = B // NCORES          # 4096
BLK = 128
NBLK = BS // BLK          # 32
NQ = 4
QS = NCT // NQ            # 25000
QR = QS + 1               # 25001 rows per on-device quarter (incl zero row)

# dma_gather index layout: "interp" = idx i at [i%16, i//16];
# "bench" = idx i at [i//(n/16), i%(n/16)] — set after HW probing.
IDX_LAYOUT = "interp"

_CACHE = {}


def _patch_dma_gather():
    """Allow 128B (fp16 row) elements: the stride field is 256B-units so
    the table rows are padded to 256B stride, but the payload read per
    descriptor is 128B."""
    import concourse.bass as Bs

    if getattr(Bs.BassGpSimd.dma_gather, "_patched_128b", False):
        return
    src = inspect.getsource(Bs.BassGpSimd.dma_gather)
    src = src.replace(
        "assert (\n            elem_size_bytes > 0 and elem_size_bytes % 256 == 0\n"
        "        )",
        "assert elem_size_bytes > 0")
    src = "def dma_gather" + src.split("def dma_gather", 1)[1]
    ns = dict(Bs.__dict__)
    exec(compile(src, "<dma_gather_128b>", "exec"), ns)
    ns["dma_gather"]._patched_128b = True
    Bs.BassGpSimd.dma_gather = ns["dma_gather"]


def _wrap_idx(flat):
    """flat [n] int16 (position order i -> partition i%128, slot i//128)
    -> wrapped [128, n/16] per the SWDGE firmware layout (16-partition
    block replicated to all 8 Q7 cores' partition groups)."""
    n = flat.shape[0]
    assert n % 16 == 0
    c = n // 16
    if IDX_LAYOUT == "interp":
        w = flat.reshape(c, 16).T          # [16, c], i at [i%16, i//16]
    else:
        w = flat.reshape(16, c)            # [16, c], i at [i//c, i%c]
    return np.ascontiguousarray(np.tile(w, (8, 1)))


def _prepare(inputs):
    """Host prep: sort+deal samples, build per-block pass schedules,
    pack int16 index arrays, aux (numf / -npad), consts. Returns
    (in_maps, sched, order)."""
    import ml_dtypes  # noqa: F401

    f = np.float32
    idx = np.asarray(inputs["cat_indices"]).astype(np.int64)
    numf = np.asarray(inputs["num_features"]).astype(f)
    table = np.asarray(inputs["embed_table"]).astype(f)

    # sorted per-sample indices (fields are exchangeable), quarter counts
    idx_s = np.sort(idx, axis=1)
    qq = idx_s // QS
    c = np.stack([(qq == k).sum(1) for k in range(NQ)], 1)      # [B, 4]
    cum = np.concatenate([np.zeros((B, 1), np.int64), np.cumsum(c, 1)], 1)

    order = np.lexsort((-c[:, 1], -c[:, 0]))                     # [B]

    # per-window (= per-block-index, shared by all cores) quarter maxes
    cw = c[order].reshape(NBLK, NCORES * BLK, NQ)
    K_qb = cw.max(1)                                             # [NBLK, 4]
    for b in range(NBLK):
        if K_qb[b].sum() % 2:
            K_qb[b, 3] += 1

    # device table: quarters of 25001 rows (last row zero), fp16,
    # rows padded to 128 fp16 (256B stride; only cols 0:64 are read)
    t16 = np.zeros((NQ * QR, 128), np.float16)
    for k in range(NQ):
        t16[k * QR:k * QR + QS, 0:64] = table[k * QS:(k + 1) * QS]

    # pack per-(block, pass) index arrays for each core
    sched = []
    tot16 = 0
    for b in range(NBLK):
        Ks = [int(K_qb[b, k]) for k in range(NQ)]
        S = sum(Ks)
        sched.append((Ks, S, S // 2, tot16))
        tot16 += S * 8
    idx16 = np.empty((NCORES, 128, tot16), np.int16)
    aux = np.zeros((NCORES, NBLK, 1, 256), np.float16)

    ar = np.arange(BLK)
    for b in range(NBLK):
        Ks, S, n_c, off = sched[b]
        w = order[b * NCORES * BLK:(b + 1) * NCORES * BLK]       # [1024]
        wc = w.reshape(BLK, NCORES)                              # [p, core]
        coff = off
        for k in range(NQ):
            Kq = Ks[k]
            if Kq == 0:
                continue
            kk = np.arange(Kq)
            # vals [p, core, Kq]
            pos = cum[wc, k][:, :, None] + kk[None, None, :]
            valid = kk[None, None, :] < c[wc, k][:, :, None]
            vals = np.take_along_axis(
                idx_s[wc], np.minimum(pos, F - 1), axis=2)
            vals = np.where(valid, vals - k * QS, QS).astype(np.int16)
            for core in range(NCORES):
                flat = vals[:, core, :].T.ravel()                # i = kk*128+p
                idx16[core, :, coff:coff + Kq * 8] = _wrap_idx(flat)
            coff += Kq * 8
        npad = (np.array(Ks)[None, None, :] - c[wc]).sum(2)      # [p, core]
        for core in range(NCORES):
            aux[core, b, 0, 0:128] = numf[wc[ar, core]]
            aux[core, b, 0, 128:256] = -npad[:, core].astype(f)

    consts = _make_consts(inputs)
    in_maps = []
    for core in range(NCORES):
        m = dict(consts)
        m["table"] = t16
        m["idx16"] = np.ascontiguousarray(idx16[core])
        m["aux"] = np.ascontiguousarray(aux[core])
        in_maps.append(m)
    return in_maps, sched, order


def _make_consts(inputs):
    f = np.float32
    h = np.float16
    ga_W = np.asarray(inputs["ga_W"]).astype(f)
    ga_b = np.asarray(inputs["ga_b"]).astype(f)
    gW = np.asarray(inputs["gW"]).astype(f)
    gb = np.asarray(inputs["gb"]).astype(f)
    lW = np.asarray(inputs["lW"]).astype(f)
    lb = np.asarray(inputs["lb"]).astype(f)
    num_W = np.asarray(inputs["num_W"]).astype(f)
    num_b = np.asarray(inputs["num_b"]).astype(f)

    W1 = ga_W * (PROBE / CD)
    # carrier bias c with c @ W1 == ga_b (exact 0 when ga_b == 0)
    cvec = np.linalg.lstsq(W1.T, ga_b, rcond=None)[0]
    gw2 = np.zeros((128, 128), h)
    gw2[0:64, 0:64] = W1.astype(h)
    gw2[64:128, 64:128] = W1.astype(h)
    dup64 = np.zeros((64, 128), h)
    dup64[np.arange(64), np.arange(64)] = 1
    dup64[np.arange(64), 64 + np.arange(64)] = 1
    i64 = np.eye(64, dtype=f)
    segf = np.vstack([np.eye(64), np.eye(64)]).astype(h)
    g0t = (gW[0] / NF).T.astype(h)
    gseg = np.vstack([g0t, g0t])
    g1aug = np.zeros((65, 64), h)
    g1aug[0:64] = (ALPHA * gW[1].T).astype(h)
    g1aug[64] = (ALPHA * gb[1]).astype(h)
    l0T = ((0.5 * lW[0]).T).astype(h)
    l1aug = np.zeros((65, 64), h)
    l1aug[0:64] = ((1 - ALPHA) * lW[1].T).astype(h)
    l1aug[64] = ((1 - ALPHA) * lb[1]).astype(h)
    cols = np.stack(
        [num_W[:, 0], num_b, gb[0], lb[0], cvec], axis=1).astype(f)
    return {
        "gw2": gw2, "dup64": dup64, "segf": segf, "gseg": gseg,
        "g0t64": np.ascontiguousarray(g0t), "g1aug": g1aug,
        "l0T": l0T, "l1aug": l1aug, "i64f": i64,
        "ident": np.eye(128, dtype=h), "ones164": np.ones((1, 64), h),
        "cols": cols,
    }


def _build(sched, tot16):
    import concourse.bass as bass
    import concourse.mybir as mybir
    import concourse.tile as tile
    from concourse import bacc, library_config
    from contextlib import ExitStack

    _patch_dma_gather()

    f16 = mybir.dt.float16
    f32 = mybir.dt.float32
    i16 = mybir.dt.int16
    AL = mybir.AluOpType
    AF = mybir.ActivationFunctionType

    nc = bacc.Bacc(None)

    table_d = nc.declare_dram_parameter("table", [NQ * QR, 128], f16,
                                        isOutput=False)
    idx_d = nc.declare_dram_parameter("idx16", [128, tot16], i16,
                                      isOutput=False)
    aux_d = nc.declare_dram_parameter("aux", [NBLK, 1, 256], f16,
                                      isOutput=False)
    gw2_d = nc.declare_dram_parameter("gw2", [128, 128], f16, isOutput=False)
    dup64_d = nc.declare_dram_parameter("dup64", [64, 128], f16, isOutput=False)
    segf_d = nc.declare_dram_parameter("segf", [128, 64], f16, isOutput=False)
    gseg_d = nc.declare_dram_parameter("gseg", [128, 64], f16, isOutput=False)
    g0t64_d = nc.declare_dram_parameter("g0t64", [64, 64], f16, isOutput=False)
    g1aug_d = nc.declare_dram_parameter("g1aug", [65, 64], f16, isOutput=False)
    l0T_d = nc.declare_dram_parameter("l0T", [64, 64], f16, isOutput=False)
    l1aug_d = nc.declare_dram_parameter("l1aug", [65, 64], f16, isOutput=False)
    i64f_d = nc.declare_dram_parameter("i64f", [64, 64], f32, isOutput=False)
    ident_d = nc.declare_dram_parameter("ident", [128, 128], f16,
                                        isOutput=False)
    ones_d = nc.declare_dram_parameter("ones164", [1, 64], f16, isOutput=False)
    cols_d = nc.declare_dram_parameter("cols", [64, 5], f32, isOutput=False)
    out_d = nc.declare_dram_parameter("out", [BS, D], f32, isOutput=True)

    with tile.TileContext(nc) as tc, ExitStack() as ctx:
        const = ctx.enter_context(tc.tile_pool(name="const", bufs=1))
        sb = ctx.enter_context(tc.tile_pool(name="sb", bufs=2))
        sbf = ctx.enter_context(tc.tile_pool(name="sbf", bufs=2))
        ptr = ctx.enter_context(tc.tile_pool(name="ptr", bufs=2, space="PSUM"))
        pu = ctx.enter_context(tc.tile_pool(name="pu", bufs=2, space="PSUM"))
        pseg = ctx.enter_context(tc.tile_pool(name="pseg", bufs=1, space="PSUM"))
        pracc = ctx.enter_context(
            tc.tile_pool(name="pracc", bufs=1, space="PSUM"))
        psm = ctx.enter_context(tc.tile_pool(name="psm", bufs=2, space="PSUM"))

        nc.gpsimd.load_library(library_config.mlp)

        def cload(dram, shape, dt, tag):
            t = const.tile(shape, dt, tag=tag)
            nc.sync.dma_start(t[:], dram[:])
            return t

        gw2_t = cload(gw2_d, [128, 128], f16, "gw2")
        dup64_t = cload(dup64_d, [64, 128], f16, "dup64")
        segf_t = cload(segf_d, [128, 64], f16, "segf")
        gseg_t = cload(gseg_d, [128, 64], f16, "gseg")
        g0t64_t = cload(g0t64_d, [64, 64], f16, "g0t64")
        g1aug_t = cload(g1aug_d, [65, 64], f16, "g1aug")
        l0T_t = cload(l0T_d, [64, 64], f16, "l0T")
        l1aug_t = cload(l1aug_d, [65, 64], f16, "l1aug")
        i64f_t = cload(i64f_d, [64, 64], f32, "i64f")
        ident_t = cload(ident_d, [128, 128], f16, "ident")
        ones_t = cload(ones_d, [1, 64], f16, "ones")
        cols_t = cload(cols_d, [64, 5], f32, "cols")
        onesrow_t = const.tile([1, 128], f16, tag="onesrow")
        nc.vector.memset(onesrow_t[:], 1.0)
        numw_c = cols_t[:, 0:1]
        numb_c = cols_t[:, 1:2]
        gb0_c = cols_t[:, 2:3]
        lb0_c = cols_t[:, 3:4]
        cc_c = cols_t[:, 4:5]

        for blk in range(NBLK):
            Ks, S, n_c, off = sched[blk]

            idxt = sb.tile([128, S * 8], i16, tag="idx")
            nc.sync.dma_start(idxt[:], idx_d[:, off:off + S * 8])
            aux = sbf.tile([1, 256], f16, tag="aux")
            nc.sync.dma_start(aux[:], aux_d[blk])

            emb = sb.tile([128, S, 64], f16, tag="emb")
            so = 0
            co = 0
            MAXSLOT = 8  # <=1024 idx per dma_gather (packed fast path)
            for k in range(NQ):
                Kq = Ks[k]
                if Kq == 0:
                    continue
                for s0 in range(0, Kq, MAXSLOT):
                    kn = min(MAXSLOT, Kq - s0)
                    nc.gpsimd.dma_gather(
                        emb[:, so + s0:so + s0 + kn, :],
                        table_d[k * QR:(k + 1) * QR, 0:64],
                        idxt[:, co + s0 * 8:co + (s0 + kn) * 8],
                        kn * 128, kn * 128, 64, elem_step=128)
                so += Kq
                co += Kq * 8

            # transposes -> [etT|sq] interleaved chunks
            ev = emb[:].rearrange("p s d -> p (s d)")
            etsq = sb.tile([128, n_c * 256], f16, tag="et")
            etv = etsq[:].rearrange("p (j c) -> p j c", c=256)
            for g0 in range(0, n_c, 8):
                gn = min(8, n_c - g0)
                trp = ptr.tile([128, 1024], f16, tag="tr")
                for j in range(gn):
                    nc.tensor.matmul(
                        out=trp[:, j * 128:(j + 1) * 128],
                        lhsT=ev[:, (g0 + j) * 128:(g0 + j + 1) * 128],
                        rhs=ident_t[:], is_transpose=True,
                        start=True, stop=True)
                nc.vector.tensor_copy(
                    out=etv[:, g0:g0 + gn, 0:128],
                    in_=trp[:, 0:gn * 128].rearrange("p (j c) -> p j c", c=128))
            # squares
            nc.vector.tensor_tensor(
                out=etv[:, :, 128:256], in0=etv[:, :, 0:128],
                in1=etv[:, :, 0:128], op=AL.mult)
            # seg: summedT | sumsqT
            seg = pseg.tile([64, 256], f32, tag="seg")
            for j in range(n_c):
                nc.tensor.matmul(
                    out=seg[:], lhsT=segf_t[:],
                    rhs=etsq[:, j * 256:(j + 1) * 256],
                    start=(j == 0), stop=(j == n_c - 1),
                    skip_group_check=True)
            # numeric embedding (transposed)
            nrep_t = psm.tile([128, 128], f32, tag="small")
            nrep = nrep_t[0:64, :]
            nc.tensor.matmul(out=nrep, lhsT=ones_t[:], rhs=aux[0:1, 0:128],
                             start=True, stop=True)
            numembT = sbf.tile([64, 128], f32, tag="numembT")
            nc.scalar.activation(out=numembT[:], in_=nrep,
                                 func=AF.Identity, bias=numb_c, scale=numw_c)
            # folds
            ssT = sbf.tile([64, 256], f32, tag="ssT")
            nc.vector.tensor_copy(out=ssT[:], in_=seg[:])
            sumT = sbf.tile([64, 128], f32, tag="sumT")
            nc.vector.tensor_tensor(out=sumT[:], in0=ssT[:, 0:128],
                                    in1=numembT[:], op=AL.add)
            # carrier = summed/PROBE + c
            car32 = sbf.tile([64, 128], f32, tag="car32")
            nc.scalar.activation(out=car32[:], in_=sumT[:], func=AF.Identity,
                                 bias=cc_c, scale=1.0 / PROBE)
            car16 = sbf.tile([64, 128], f16, tag="car16")
            nc.vector.tensor_copy(out=car16[:], in_=car32[:])
            pcar = psm.tile([128, 128], f32, tag="small")
            nc.tensor.matmul(out=pcar[:], lhsT=dup64_t[:], rhs=car16[:],
                             start=True, stop=True)
            car2 = sbf.tile([128, 128], f16, tag="car2")
            nc.vector.tensor_copy(out=car2[:], in_=pcar[:])
            # z-add in place on et cols
            nc.vector.tensor_tensor(
                out=etv[:, :, 0:128], in0=etv[:, :, 0:128],
                in1=car2[:].rearrange("p (o n) -> p o n", o=1)
                .to_broadcast([128, n_c, 128]),
                op=AL.add)
            # u matmuls + relu evict
            r16 = sb.tile([128, n_c * 128], f16, tag="r16")
            for g0 in range(0, n_c, 4):
                gn = min(4, n_c - g0)
                up = pu.tile([128, 512], f32, tag="u")
                for j in range(gn):
                    nc.tensor.matmul(
                        out=up[:, j * 128:(j + 1) * 128], lhsT=gw2_t[:],
                        rhs=etv[:, g0 + j:g0 + j + 1, 0:128], start=True, stop=True)
                nc.scalar.activation(
                    out=r16[:, g0 * 128:(g0 + gn) * 128],
                    in_=up[:, 0:gn * 128], func=AF.Relu)
            # numeric field u
            znum = sbf.tile([64, 128], f16, tag="znum")
            nc.vector.tensor_tensor(out=znum[:], in0=numembT[:],
                                    in1=car32[:], op=AL.add)
            unum_t = psm.tile([128, 128], f32, tag="small")
            unum = unum_t[0:64, :]
            nc.tensor.matmul(out=unum, lhsT=gw2_t[0:64, 0:64],
                             rhs=znum[:], start=True, stop=True)
            rnum16 = sbf.tile([64, 128], f16, tag="rnum16")
            nc.scalar.activation(out=rnum16[:], in_=unum, func=AF.Relu)
            # pad correction: -npad * relu(t),  t = carrier @ W1
            pt_t = psm.tile([128, 128], f32, tag="small")
            pt = pt_t[0:64, :]
            nc.tensor.matmul(out=pt, lhsT=gw2_t[0:64, 0:64],
                             rhs=car16[:], start=True, stop=True)
            tr16 = sbf.tile([64, 128], f16, tag="tr16")
            nc.scalar.activation(out=tr16[:], in_=pt, func=AF.Relu)
            pn_t = psm.tile([128, 128], f32, tag="small")
            pn = pn_t[0:64, :]
            nc.tensor.matmul(out=pn, lhsT=ones_t[:], rhs=aux[0:1, 128:256],
                             start=True, stop=True)
            npad64 = sbf.tile([64, 128], f16, tag="npad64")
            nc.vector.tensor_copy(out=npad64[:], in_=pn)
            trs16 = sbf.tile([64, 128], f16, tag="trs16")
            nc.vector.tensor_tensor(out=trs16[:], in0=tr16[:],
                                    in1=npad64[:], op=AL.mult)
            # racc: g1^T accumulation (gW0/NF folded into lhsT)
            gacc = pracc.tile([64, 128], f32, tag="racc")
            for j in range(n_c):
                nc.tensor.matmul(
                    out=gacc[:], lhsT=gseg_t[:],
                    rhs=r16[:, j * 128:(j + 1) * 128],
                    start=(j == 0), stop=False, skip_group_check=True)
            nc.tensor.matmul(out=gacc[:], lhsT=g0t64_t[:], rhs=rnum16[:],
                             start=False, stop=False, skip_group_check=True)
            nc.tensor.matmul(out=gacc[:], lhsT=g0t64_t[:], rhs=trs16[:],
                             start=False, stop=True, skip_group_check=True)
            h1aug = sbf.tile([65, 128], f16, tag="h1aug")
            nc.scalar.activation(out=h1aug[0:64, :], in_=gacc[:],
                                 func=AF.Relu, bias=gb0_c)
            nc.vector.tensor_copy(out=h1aug[64:65, :], in_=onesrow_t[:])
            # local branch
            lsq = sbf.tile([64, 128], f32, tag="lsq")
            nc.vector.tensor_tensor(out=lsq[:], in0=sumT[:], in1=sumT[:],
                                    op=AL.mult)
            lT16 = sbf.tile([64, 128], f16, tag="lT16")
            nc.vector.tensor_tensor(out=lT16[:], in0=lsq[:],
                                    in1=ssT[:, 128:256], op=AL.subtract)
            l1p_t = psm.tile([128, 128], f32, tag="small")
            l1p = l1p_t[0:64, :]
            nc.tensor.matmul(out=l1p, lhsT=l0T_t[:], rhs=lT16[:],
                             start=True, stop=True)
            l1aug = sbf.tile([65, 128], f16, tag="l1aug")
            nc.scalar.activation(out=l1aug[0:64, :], in_=l1p,
                                 func=AF.Relu, bias=lb0_c)
            nc.vector.tensor_copy(out=l1aug[64:65, :], in_=onesrow_t[:])
            # combine + final transpose
            outp_t = psm.tile([128, 128], f32, tag="small")
            outp = outp_t[0:64, :]
            nc.tensor.matmul(out=outp, lhsT=g1aug_t[:], rhs=h1aug[:],
                             start=True, stop=False, skip_group_check=True)
            nc.tensor.matmul(out=outp, lhsT=l1aug_t[:], rhs=l1aug[:],
                             start=False, stop=True, skip_group_check=True)
            outT = sbf.tile([64, 128], f32, tag="outT")
            nc.vector.tensor_copy(out=outT[:], in_=outp)
            finp_t = psm.tile([128, 128], f32, tag="small")
            finp = finp_t[:, 0:64]
            nc.tensor.matmul(out=finp, lhsT=outT[:], rhs=i64f_t[:],
                             is_transpose=True, start=True, stop=True)
            orow = sbf.tile([128, 64], f32, tag="orow")
            nc.vector.tensor_copy(out=orow[:], in_=finp)
            nc.sync.dma_start(out_d[blk * BLK:(blk + 1) * BLK, :], orow[:])

    return nc


def _get_nc(sched, tot16):
    key = ("nc", tuple((tuple(s[0]), s[1], s[2], s[3]) for s in sched))
    if _CACHE.get("key") != key:
        print("[kernel] building bass module...", flush=True)
        nc = _build(sched, tot16)
        nc.finalize()
        _CACHE["nc"] = nc
        _CACHE["key"] = key
        print("[kernel] build done", flush=True)
    return _CACHE["nc"]


def kernel(cat_indices, num_features, embed_table, num_W, num_b,
           ga_W, ga_b, gW, gb, lW, lb):
    from concourse.bass_utils import run_bass_kernel_spmd

    inputs = dict(
        cat_indices=cat_indices, num_features=num_features,
        embed_table=embed_table, num_W=num_W, num_b=num_b,
        ga_W=ga_W, ga_b=ga_b, gW=gW, gb=gb, lW=lW, lb=lb)
    in_maps, sched, order = _prepare(inputs)
    tot16 = in_maps[0]["idx16"].shape[1]
    nc = _get_nc(sched, tot16)

    print("[kernel] launching spmd run...", flush=True)
    res = run_bass_kernel_spmd(nc, in_maps, list(range(NCORES)))
    print("[kernel] run complete", flush=True)

    out = np.empty((B, D), np.float32)
    rows = np.arange(BS)
    base = (rows // BLK) * (NCORES * BLK) + (rows % BLK) * NCORES
    for core in range(NCORES):
        out[order[base + core]] = np.asarray(
            res.results[core]["out"]).astype(np.float32)
    return out


# revision 13
# speedup vs baseline: 1.3710x; 1.3710x over previous
"""Trainium2 Bass kernel for nn_CatEmbedder (gnn_message_passing).

Strategy (v2):
- Batch-parallel across 8 cores (4096 samples each), table replicated.
- Gather via dma_gather (one SWDGE instruction per range-pass per block)
  instead of 1600 per-field indirect DMAs (which serialized ~1.8ms on Q7).
  dma_gather needs int16 indices, so the 100k-row table is split into 4
  quarters of 25001 rows (each with a trailing zero row for padding) and
  each block issues 4 gathers. Per-sample quarter-counts vary, so samples
  are globally sorted by count profile and dealt round-robin to cores;
  each block pads every sample to the block's per-quarter max with
  zero-row lookups. Padded slots contribute relu(t_s) to the global
  branch, which is subtracted exactly via a per-sample pad-count
  correction term.
- fp16 data path everywhere (4x faster matmuls than fp32, half the
  gather bytes); fp32 PSUM accumulation. rel err ~4e-4 vs 2e-2 budget.
- Algebraic fold: gacc_f = (e_f + summed/PROBE) @ (ga_W*PROBE/CD) + ga_b,
  so the per-field bias becomes one broadcast add on the embeddings.
"""

import sys
import types
import inspect

import numpy as np

sys.path.insert(0, "/opt/trn_rl_repo")

# ---- problem constants ----
B, F, D, NCT = 32768, 50, 64, 100000
PROBE, ALPHA = 39.0, 0.5
NF = F + 1
CD = NF + PROBE
NCORES = 8
BS = B // NCORES          # 4096
BLK = 128
NBLK = BS // BLK          # 32
NQ = 4
QS = NCT // NQ            # 25000
QR = QS + 1               # 25001 rows per on-device quarter (incl zero row)

# dma_gather index layout: "interp" = idx i at [i%16, i//16];
# "bench" = idx i at [i//(n/16), i%(n/16)] — set after HW probing.
IDX_LAYOUT = "interp"

_CACHE = {}


def _patch_dma_gather():
    """Allow 128B (fp16 row) elements: the stride field is 256B-units so
    the table rows are padded to 256B stride, but the payload read per
    descriptor is 128B."""
    import concourse.bass as Bs

    if getattr(Bs.BassGpSimd.dma_gather, "_patched_128b", False):
        return
    src = inspect.getsource(Bs.BassGpSimd.dma_gather)
    src = src.replace(
        "assert (\n            elem_size_bytes > 0 and elem_size_bytes % 256 == 0\n"
        "        )",
        "assert elem_size_bytes > 0")
    src = "def dma_gather" + src.split("def dma_gather", 1)[1]
    ns = dict(Bs.__dict__)
    exec(compile(src, "<dma_gather_128b>", "exec"), ns)
    ns["dma_gather"]._patched_128b = True
    Bs.BassGpSimd.dma_gather = ns["dma_gather"]


def _wrap_idx(flat):
    """flat [n] int16 (position order i -> partition i%128, slot i//128)
    -> wrapped [128, n/16] per the SWDGE firmware layout (16-partition
    block replicated to all 8 Q7 cores' partition groups)."""
    n = flat.shape[0]
    assert n % 16 == 0
    c = n // 16
    if IDX_LAYOUT == "interp":
        w = flat.reshape(c, 16).T          # [16, c], i at [i%16, i//16]
    else:
        w = flat.reshape(16, c)            # [16, c], i at [i//c, i%c]
    return np.ascontiguousarray(np.tile(w, (8, 1)))


def _prepare(inputs):
    """Host prep: sort+deal samples, build per-block pass schedules,
    pack int16 index arrays, aux (numf / -npad), consts. Returns
    (in_maps, sched, order)."""
    import ml_dtypes  # noqa: F401

    f = np.float32
    idx = np.asarray(inputs["cat_indices"]).astype(np.int64)
    numf = np.asarray(inputs["num_features"]).astype(f)
    table = np.asarray(inputs["embed_table"]).astype(f)

    order = np.arange(B)
    t16 = np.ascontiguousarray(table.astype(np.float16))

    sched = [([F, 0, 0, 0], F, F // 2, 0) for _ in range(NBLK)]
    aux = np.zeros((NCORES, NBLK, 1, 256), np.float16)
    idx32 = np.empty((NCORES, BS, F), np.int32)
    ar = np.arange(BLK)
    for b in range(NBLK):
        w = order[b * NCORES * BLK:(b + 1) * NCORES * BLK]
        wc = w.reshape(BLK, NCORES)
        for core in range(NCORES):
            aux[core, b, 0, 0:128] = numf[wc[ar, core]]
            idx32[core, b * BLK:(b + 1) * BLK, :] = idx[wc[ar, core]]

    consts = _make_consts(inputs)
    in_maps = []
    for core in range(NCORES):
        m = dict(consts)
        m["table"] = t16
        m["idx32"] = np.ascontiguousarray(idx32[core])
        m["aux"] = np.ascontiguousarray(aux[core])
        in_maps.append(m)
    return in_maps, sched, order


def _make_consts(inputs):
    f = np.float32
    h = np.float16
    ga_W = np.asarray(inputs["ga_W"]).astype(f)
    ga_b = np.asarray(inputs["ga_b"]).astype(f)
    gW = np.asarray(inputs["gW"]).astype(f)
    gb = np.asarray(inputs["gb"]).astype(f)
    lW = np.asarray(inputs["lW"]).astype(f)
    lb = np.asarray(inputs["lb"]).astype(f)
    num_W = np.asarray(inputs["num_W"]).astype(f)
    num_b = np.asarray(inputs["num_b"]).astype(f)

    W1 = ga_W * (PROBE / CD)
    # carrier bias c with c @ W1 == ga_b (exact 0 when ga_b == 0)
    cvec = np.linalg.lstsq(W1.T, ga_b, rcond=None)[0]
    gw2 = np.zeros((128, 128), h)
    gw2[0:64, 0:64] = W1.astype(h)
    gw2[64:128, 64:128] = W1.astype(h)
    dup64 = np.zeros((64, 128), h)
    dup64[np.arange(64), np.arange(64)] = 1
    dup64[np.arange(64), 64 + np.arange(64)] = 1
    i64 = np.eye(64, dtype=f)
    segf = np.vstack([np.eye(64), np.eye(64)]).astype(h)
    g0t = (gW[0] / NF).T.astype(h)
    gseg = np.vstack([g0t, g0t])
    g1aug = np.zeros((65, 64), h)
    g1aug[0:64] = (ALPHA * gW[1].T).astype(h)
    g1aug[64] = (ALPHA * gb[1]).astype(h)
    l0T = ((0.5 * lW[0]).T).astype(h)
    l1aug = np.zeros((65, 64), h)
    l1aug[0:64] = ((1 - ALPHA) * lW[1].T).astype(h)
    l1aug[64] = ((1 - ALPHA) * lb[1]).astype(h)
    cols = np.stack(
        [num_W[:, 0], num_b, gb[0], lb[0], cvec], axis=1).astype(f)
    return {
        "gw2": gw2, "dup64": dup64, "segf": segf, "gseg": gseg,
        "g0t64": np.ascontiguousarray(g0t), "g1aug": g1aug,
        "l0T": l0T, "l1aug": l1aug, "i64f": i64,
        "ident": np.eye(128, dtype=h), "ones164": np.ones((1, 64), h),
        "cols": cols,
    }


def _build(sched, tot16):
    import concourse.bass as bass
    import concourse.mybir as mybir
    import concourse.tile as tile
    from concourse import bacc
    from contextlib import ExitStack

    f16 = mybir.dt.float16
    f32 = mybir.dt.float32
    i16 = mybir.dt.int16
    AL = mybir.AluOpType
    AF = mybir.ActivationFunctionType

    nc = bacc.Bacc(None)

    table_d = nc.declare_dram_parameter("table", [NCT, 64], f16,
                                        isOutput=False)
    idx_d = nc.declare_dram_parameter("idx32", [BS, F], mybir.dt.int32,
                                      isOutput=False)
    aux_d = nc.declare_dram_parameter("aux", [NBLK, 1, 256], f16,
                                      isOutput=False)
    gw2_d = nc.declare_dram_parameter("gw2", [128, 128], f16, isOutput=False)
    dup64_d = nc.declare_dram_parameter("dup64", [64, 128], f16, isOutput=False)
    segf_d = nc.declare_dram_parameter("segf", [128, 64], f16, isOutput=False)
    gseg_d = nc.declare_dram_parameter("gseg", [128, 64], f16, isOutput=False)
    g0t64_d = nc.declare_dram_parameter("g0t64", [64, 64], f16, isOutput=False)
    g1aug_d = nc.declare_dram_parameter("g1aug", [65, 64], f16, isOutput=False)
    l0T_d = nc.declare_dram_parameter("l0T", [64, 64], f16, isOutput=False)
    l1aug_d = nc.declare_dram_parameter("l1aug", [65, 64], f16, isOutput=False)
    i64f_d = nc.declare_dram_parameter("i64f", [64, 64], f32, isOutput=False)
    ident_d = nc.declare_dram_parameter("ident", [128, 128], f16,
                                        isOutput=False)
    ones_d = nc.declare_dram_parameter("ones164", [1, 64], f16, isOutput=False)
    cols_d = nc.declare_dram_parameter("cols", [64, 5], f32, isOutput=False)
    out_d = nc.declare_dram_parameter("out", [BS, D], f32, isOutput=True)

    with tile.TileContext(nc) as tc, ExitStack() as ctx:
        const = ctx.enter_context(tc.tile_pool(name="const", bufs=1))
        sb = ctx.enter_context(tc.tile_pool(name="sb", bufs=2))
        sbf = ctx.enter_context(tc.tile_pool(name="sbf", bufs=2))
        ptr = ctx.enter_context(tc.tile_pool(name="ptr", bufs=2, space="PSUM"))
        pu = ctx.enter_context(tc.tile_pool(name="pu", bufs=2, space="PSUM"))
        pseg = ctx.enter_context(tc.tile_pool(name="pseg", bufs=1, space="PSUM"))
        pracc = ctx.enter_context(
            tc.tile_pool(name="pracc", bufs=1, space="PSUM"))
        psm = ctx.enter_context(tc.tile_pool(name="psm", bufs=2, space="PSUM"))

        def cload(dram, shape, dt, tag):
            t = const.tile(shape, dt, tag=tag)
            nc.sync.dma_start(t[:], dram[:])
            return t

        gw2_t = cload(gw2_d, [128, 128], f16, "gw2")
        dup64_t = cload(dup64_d, [64, 128], f16, "dup64")
        segf_t = cload(segf_d, [128, 64], f16, "segf")
        gseg_t = cload(gseg_d, [128, 64], f16, "gseg")
        g0t64_t = cload(g0t64_d, [64, 64], f16, "g0t64")
        g1aug_t = cload(g1aug_d, [65, 64], f16, "g1aug")
        l0T_t = cload(l0T_d, [64, 64], f16, "l0T")
        l1aug_t = cload(l1aug_d, [65, 64], f16, "l1aug")
        i64f_t = cload(i64f_d, [64, 64], f32, "i64f")
        ident_t = cload(ident_d, [128, 128], f16, "ident")
        ones_t = cload(ones_d, [1, 64], f16, "ones")
        cols_t = cload(cols_d, [64, 5], f32, "cols")
        onesrow_t = const.tile([1, 128], f16, tag="onesrow")
        nc.vector.memset(onesrow_t[:], 1.0)
        numw_c = cols_t[:, 0:1]
        numb_c = cols_t[:, 1:2]
        gb0_c = cols_t[:, 2:3]
        lb0_c = cols_t[:, 3:4]
        cc_c = cols_t[:, 4:5]

        for blk in range(NBLK):
            Ks, S, n_c, off = sched[blk]

            idxt = sb.tile([128, F], mybir.dt.int32, tag="idx")
            nc.sync.dma_start(idxt[:], idx_d[blk * BLK:(blk + 1) * BLK, :])
            aux = sbf.tile([1, 256], f16, tag="aux")
            nc.sync.dma_start(aux[:], aux_d[blk])

            emb = sb.tile([128, S * 64], f16, tag="emb")
            for ff in range(F):
                nc.gpsimd.indirect_dma_start(
                    out=emb[:, ff * 64:(ff + 1) * 64],
                    out_offset=None,
                    in_=table_d[:, :],
                    in_offset=bass.IndirectOffsetOnAxis(
                        ap=idxt[:, ff:ff + 1], axis=0))

            # transposes -> [etT|sq] interleaved chunks
            ev = emb[:]
            etsq = sb.tile([128, n_c * 256], f16, tag="et")
            etv = etsq[:].rearrange("p (j c) -> p j c", c=256)
            for g0 in range(0, n_c, 8):
                gn = min(8, n_c - g0)
                trp = ptr.tile([128, 1024], f16, tag="tr")
                for j in range(gn):
                    nc.tensor.matmul(
                        out=trp[:, j * 128:(j + 1) * 128],
                        lhsT=ev[:, (g0 + j) * 128:(g0 + j + 1) * 128],
                        rhs=ident_t[:], is_transpose=True,
                        start=True, stop=True)
                nc.vector.tensor_copy(
                    out=etv[:, g0:g0 + gn, 0:128],
                    in_=trp[:, 0:gn * 128].rearrange("p (j c) -> p j c", c=128))
            # squares
            nc.vector.tensor_tensor(
                out=etv[:, :, 128:256], in0=etv[:, :, 0:128],
                in1=etv[:, :, 0:128], op=AL.mult)
            # seg: summedT | sumsqT
            seg = pseg.tile([64, 256], f32, tag="seg")
            for j in range(n_c):
                nc.tensor.matmul(
                    out=seg[:], lhsT=segf_t[:],
                    rhs=etsq[:, j * 256:(j + 1) * 256],
                    start=(j == 0), stop=(j == n_c - 1),
                    skip_group_check=True)
            # numeric embedding (transposed)
            nrep_t = psm.tile([128, 128], f32, tag="small")
            nrep = nrep_t[0:64, :]
            nc.tensor.matmul(out=nrep, lhsT=ones_t[:], rhs=aux[0:1, 0:128],
                             start=True, stop=True)
            numembT = sbf.tile([64, 128], f32, tag="numembT")
            nc.scalar.activation(out=numembT[:], in_=nrep,
                                 func=AF.Identity, bias=numb_c, scale=numw_c)
            # folds
            ssT = sbf.tile([64, 256], f32, tag="ssT")
            nc.vector.tensor_copy(out=ssT[:], in_=seg[:])
            sumT = sbf.tile([64, 128], f32, tag="sumT")
            nc.vector.tensor_tensor(out=sumT[:], in0=ssT[:, 0:128],
                                    in1=numembT[:], op=AL.add)
            # carrier = summed/PROBE + c
            car32 = sbf.tile([64, 128], f32, tag="car32")
            nc.scalar.activation(out=car32[:], in_=sumT[:], func=AF.Identity,
                                 bias=cc_c, scale=1.0 / PROBE)
            car16 = sbf.tile([64, 128], f16, tag="car16")
            nc.vector.tensor_copy(out=car16[:], in_=car32[:])
            pcar = psm.tile([128, 128], f32, tag="small")
            nc.tensor.matmul(out=pcar[:], lhsT=dup64_t[:], rhs=car16[:],
                             start=True, stop=True)
            car2 = sbf.tile([128, 128], f16, tag="car2")
            nc.vector.tensor_copy(out=car2[:], in_=pcar[:])
            # z-add in place on et cols
            nc.vector.tensor_tensor(
                out=etv[:, :, 0:128], in0=etv[:, :, 0:128],
                in1=car2[:].rearrange("p (o n) -> p o n", o=1)
                .to_broadcast([128, n_c, 128]),
                op=AL.add)
            # u matmuls + relu evict
            r16 = sb.tile([128, n_c * 128], f16, tag="r16")
            for g0 in range(0, n_c, 4):
                gn = min(4, n_c - g0)
                up = pu.tile([128, 512], f32, tag="u")
                for j in range(gn):
                    nc.tensor.matmul(
                        out=up[:, j * 128:(j + 1) * 128], lhsT=gw2_t[:],
                        rhs=etv[:, g0 + j:g0 + j + 1, 0:128], start=True, stop=True)
                nc.scalar.activation(
                    out=r16[:, g0 * 128:(g0 + gn) * 128],
                    in_=up[:, 0:gn * 128], func=AF.Relu)
            # numeric field u
            znum = sbf.tile([64, 128], f16, tag="znum")
            nc.vector.tensor_tensor(out=znum[:], in0=numembT[:],
                                    in1=car32[:], op=AL.add)
            unum_t = psm.tile([128, 128], f32, tag="small")
            unum = unum_t[0:64, :]
            nc.tensor.matmul(out=unum, lhsT=gw2_t[0:64, 0:64],
                             rhs=znum[:], start=True, stop=True)
            rnum16 = sbf.tile([64, 128], f16, tag="rnum16")
            nc.scalar.activation(out=rnum16[:], in_=unum, func=AF.Relu)
            # pad correction: -npad * relu(t),  t = carrier @ W1
            pt_t = psm.tile([128, 128], f32, tag="small")
            pt = pt_t[0:64, :]
            nc.tensor.matmul(out=pt, lhsT=gw2_t[0:64, 0:64],
                             rhs=car16[:], start=True, stop=True)
            tr16 = sbf.tile([64, 128], f16, tag="tr16")
            nc.scalar.activation(out=tr16[:], in_=pt, func=AF.Relu)
            pn_t = psm.tile([128, 128], f32, tag="small")
            pn = pn_t[0:64, :]
            nc.tensor.matmul(out=pn, lhsT=ones_t[:], rhs=aux[0:1, 128:256],
                             start=True, stop=True)
            npad64 = sbf.tile([64, 128], f16, tag="npad64")
            nc.vector.tensor_copy(out=npad64[:], in_=pn)
            trs16 = sbf.tile([64, 128], f16, tag="trs16")
            nc.vector.tensor_tensor(out=trs16[:], in0=tr16[:],
                                    in1=npad64[:], op=AL.mult)
            # racc: g1^T accumulation (gW0/NF folded into lhsT)
            gacc = pracc.tile([64, 128], f32, tag="racc")
            for j in range(n_c):
                nc.tensor.matmul(
                    out=gacc[:], lhsT=gseg_t[:],
                    rhs=r16[:, j * 128:(j + 1) * 128],
                    start=(j == 0), stop=False, skip_group_check=True)
            nc.tensor.matmul(out=gacc[:], lhsT=g0t64_t[:], rhs=rnum16[:],
                             start=False, stop=False, skip_group_check=True)
            nc.tensor.matmul(out=gacc[:], lhsT=g0t64_t[:], rhs=trs16[:],
                             start=False, stop=True, skip_group_check=True)
            h1aug = sbf.tile([65, 128], f16, tag="h1aug")
            nc.scalar.activation(out=h1aug[0:64, :], in_=gacc[:],
                                 func=AF.Relu, bias=gb0_c)
            nc.vector.tensor_copy(out=h1aug[64:65, :], in_=onesrow_t[:])
            # local branch
            lsq = sbf.tile([64, 128], f32, tag="lsq")
            nc.vector.tensor_tensor(out=lsq[:], in0=sumT[:], in1=sumT[:],
                                    op=AL.mult)
            lT16 = sbf.tile([64, 128], f16, tag="lT16")
            nc.vector.tensor_tensor(out=lT16[:], in0=lsq[:],
                                    in1=ssT[:, 128:256], op=AL.subtract)
            l1p_t = psm.tile([128, 128], f32, tag="small")
            l1p = l1p_t[0:64, :]
            nc.tensor.matmul(out=l1p, lhsT=l0T_t[:], rhs=lT16[:],
                             start=True, stop=True)
            l1aug = sbf.tile([65, 128], f16, tag="l1aug")
            nc.scalar.activation(out=l1aug[0:64, :], in_=l1p,
                                 func=AF.Relu, bias=lb0_c)
            nc.vector.tensor_copy(out=l1aug[64:65, :], in_=onesrow_t[:])
            # combine + final transpose
            outp_t = psm.tile([128, 128], f32, tag="small")
            outp = outp_t[0:64, :]
            nc.tensor.matmul(out=outp, lhsT=g1aug_t[:], rhs=h1aug[:],
                             start=True, stop=False, skip_group_check=True)
            nc.tensor.matmul(out=outp, lhsT=l1aug_t[:], rhs=l1aug[:],
                             start=False, stop=True, skip_group_check=True)
            outT = sbf.tile([64, 128], f32, tag="outT")
            nc.vector.tensor_copy(out=outT[:], in_=outp)
            finp_t = psm.tile([128, 128], f32, tag="small")
            finp = finp_t[:, 0:64]
            nc.tensor.matmul(out=finp, lhsT=outT[:], rhs=i64f_t[:],
                             is_transpose=True, start=True, stop=True)
            orow = sbf.tile([128, 64], f32, tag="orow")
            nc.vector.tensor_copy(out=orow[:], in_=finp)
            nc.sync.dma_start(out_d[blk * BLK:(blk + 1) * BLK, :], orow[:])

    return nc


def _get_nc(sched, tot16):
    key = ("nc", tuple((tuple(s[0]), s[1], s[2], s[3]) for s in sched))
    if _CACHE.get("key") != key:
        print("[kernel] building bass module...", flush=True)
        nc = _build(sched, tot16)
        nc.finalize()
        _CACHE["nc"] = nc
        _CACHE["key"] = key
        print("[kernel] build done", flush=True)
    return _CACHE["nc"]


def kernel(cat_indices, num_features, embed_table, num_W, num_b,
           ga_W, ga_b, gW, gb, lW, lb):
    from concourse.bass_utils import run_bass_kernel_spmd

    inputs = dict(
        cat_indices=cat_indices, num_features=num_features,
        embed_table=embed_table, num_W=num_W, num_b=num_b,
        ga_W=ga_W, ga_b=ga_b, gW=gW, gb=gb, lW=lW, lb=lb)
    in_maps, sched, order = _prepare(inputs)
    tot16 = 0
    nc = _get_nc(sched, tot16)

    print("[kernel] launching spmd run...", flush=True)
    res = run_bass_kernel_spmd(nc, in_maps, list(range(NCORES)))
    print("[kernel] run complete", flush=True)

    out = np.empty((B, D), np.float32)
    rows = np.arange(BS)
    base = (rows // BLK) * (NCORES * BLK) + (rows % BLK) * NCORES
    for core in range(NCORES):
        out[order[base + core]] = np.asarray(
            res.results[core]["out"]).astype(np.float32)
    return out


# revision 15
# speedup vs baseline: 1.3910x; 1.0146x over previous
"""Trainium2 Bass kernel for nn_CatEmbedder (gnn_message_passing).

Takes FULL inputs, shards batch B=32768 across 8 NeuronCores (4096 each),
replicates the embedding table + weights, runs an SPMD Bass kernel, and
concatenates the per-core outputs.

Per-core pipeline (32 blocks x 128 samples):
  1. indirect-DMA gather: emb[p, f*64:(f+1)*64] = table[idx[p,f]]  ([128,3200])
  2. PE transposes of [128,128] chunks (2 fields each) -> PSUM -> SBUF (et)
  3. squares (ACT/DVE split) into interleaved sq chunks
  4. PE seg-matmuls: field-sum + field-sum-of-squares  -> [64,256] PSUM
  5. PE u-matmuls: u_f = (S + PROBE*support_f)/c + ga_b  (bias via aug row)
  6. ACT relu-evict (bf16) -> PE accumulates sum_f relu(u_f)
  7. transposed MLPs for global/local branches, combine, transpose back, store
"""

import os
import sys
import numpy as np

sys.path.insert(0, "/opt/trn_rl_repo")

# ---- problem constants (hardcoded per the contract) ----
B, F, D, NCT = 32768, 50, 64, 100000
PROBE, ALPHA = 39.0, 0.5
NF = F + 1              # 51 fields
CD = NF + PROBE         # 90.0
NCORES = 8
BS = B // NCORES        # 4096 samples per core
BLK = 128
SUPER = 8               # idx/numf superblock (blocks per DMA)
NBLK_FULL = BS // BLK   # 32

USE_F32R = False        # fast fp32 matmul mode (walrus needs fp32r-typed producers)

_CACHE = {}


def _build(nblk=NBLK_FULL, reps=1):
    import concourse.bass as bass
    import concourse.mybir as mybir
    import concourse.tile as tile
    from concourse import bacc
    from contextlib import ExitStack

    f32 = mybir.dt.float32
    f32r = mybir.dt.float32r
    bf16 = mybir.dt.bfloat16
    i32 = mybir.dt.int32
    AL = mybir.AluOpType
    AF = mybir.ActivationFunctionType

    def r(ap):  # fp32 -> fp32r view for fast matmuls
        return ap.bitcast(f32r) if USE_F32R else ap

    nc = bacc.Bacc(None)

    # ---- DRAM parameters (order matters only for debug; bound by name) ----
    idx_d = nc.declare_dram_parameter("cat_idx", [BS, F], i32, isOutput=False)
    numf_d = nc.declare_dram_parameter("numf", [BS], f32, isOutput=False)
    table_d = nc.declare_dram_parameter("table", [NCT, D], f32, isOutput=False)
    ident_d = nc.declare_dram_parameter("ident128", [128, 128], f32, isOutput=False)
    segf_d = nc.declare_dram_parameter("seg_f", [128, D], f32, isOutput=False)
    segb_d = nc.declare_dram_parameter("seg_b", [128, D], bf16, isOutput=False)
    i64b_d = nc.declare_dram_parameter("i64_b", [D, D], bf16, isOutput=False)
    i64f_d = nc.declare_dram_parameter("i64_f", [D, D], f32, isOutput=False)
    waug_d = nc.declare_dram_parameter("waug", [D + 1, 128], f32, isOutput=False)
    gw2_d = nc.declare_dram_parameter("gw2", [128, 128], f32, isOutput=False)
    g0_d = nc.declare_dram_parameter("g0T", [D, D], f32, isOutput=False)
    g1_d = nc.declare_dram_parameter("g1aug", [D + 1, D], f32, isOutput=False)
    l0_d = nc.declare_dram_parameter("l0T", [D, D], f32, isOutput=False)
    l1_d = nc.declare_dram_parameter("l1aug", [D + 1, D], f32, isOutput=False)
    cols_d = nc.declare_dram_parameter("cols", [D, 4], f32, isOutput=False)
    ones_d = nc.declare_dram_parameter("ones164", [1, D], f32, isOutput=False)
    out_d = nc.declare_dram_parameter("out", [BS, D], f32, isOutput=True)

    GROUPS = [(0, 4), (4, 4), (8, 4), (12, 4), (16, 4), (20, 4), (24, 1)]
    NCHUNK = 25  # 25 chunks of 128 cols (2 fields each)

    with tile.TileContext(nc) as tc, ExitStack() as ctx:
        const = ctx.enter_context(tc.tile_pool(name="const", bufs=1))
        sb = ctx.enter_context(tc.tile_pool(name="sb", bufs=3))
        pst = ctx.enter_context(tc.tile_pool(name="pst", bufs=2, space="PSUM"))
        psu = ctx.enter_context(tc.tile_pool(name="psu", bufs=2, space="PSUM"))
        pseg = ctx.enter_context(tc.tile_pool(name="pseg", bufs=1, space="PSUM"))
        pracc = ctx.enter_context(tc.tile_pool(name="pracc", bufs=1, space="PSUM"))
        psm = ctx.enter_context(tc.tile_pool(name="psm", bufs=2, space="PSUM"))

        # ---- load constants once ----
        ident_t = const.tile([128, 128], f32)
        nc.sync.dma_start(ident_t[:], ident_d[:])
        segf_t = const.tile([128, D], f32)
        nc.sync.dma_start(segf_t[:], segf_d[:])
        segb_t = const.tile([128, D], bf16)
        nc.sync.dma_start(segb_t[:], segb_d[:])
        i64b_t = const.tile([D, D], bf16)
        nc.sync.dma_start(i64b_t[:], i64b_d[:])
        i64f_t = const.tile([D, D], f32)
        nc.sync.dma_start(i64f_t[:], i64f_d[:])
        waug_t = const.tile([D + 1, 128], f32)
        nc.sync.dma_start(waug_t[:], waug_d[:])
        gw2_t = const.tile([128, 128], f32)
        nc.sync.dma_start(gw2_t[:], gw2_d[:])
        g0_t = const.tile([D, D], f32)
        nc.sync.dma_start(g0_t[:], g0_d[:])
        g1_t = const.tile([D + 1, D], f32)
        nc.sync.dma_start(g1_t[:], g1_d[:])
        l0_t = const.tile([D, D], f32)
        nc.sync.dma_start(l0_t[:], l0_d[:])
        l1_t = const.tile([D + 1, D], f32)
        nc.sync.dma_start(l1_t[:], l1_d[:])
        cols_t = const.tile([D, 4], f32)
        nc.sync.dma_start(cols_t[:], cols_d[:])
        ones_t = const.tile([1, D], f32)
        nc.sync.dma_start(ones_t[:], ones_d[:])
        onesrow_t = const.tile([1, 128], f32)
        nc.vector.memset(onesrow_t[:], 1.0)

        numw_c = cols_t[:, 0:1]
        numb_c = cols_t[:, 1:2]
        gb0_c = cols_t[:, 2:3]
        lb0_c = cols_t[:, 3:4]

        idx_view = idx_d[:].rearrange("(s k p) f -> s p k f", p=BLK, k=SUPER)

        idx_t = None
        numf_t = None
        rep_cm = tc.For_i(0, reps, 1) if reps > 1 else None
        if rep_cm is not None:
            rep_cm.__enter__()
        for blk in range(nblk):
            s = blk % SUPER
            if s == 0:
                si = blk // SUPER
                idx_t = sb.tile([128, SUPER * F], i32, tag="idx")
                nc.sync.dma_start(
                    idx_t[:].rearrange("p (k f) -> p k f", k=SUPER), idx_view[si]
                )
                numf_t = sb.tile([1, SUPER * BLK], f32, tag="numf")
                nc.sync.dma_start(
                    numf_t[:], numf_d[None, si * SUPER * BLK:(si + 1) * SUPER * BLK]
                )

            # ---- 1. gather (one indirect DMA per field: HW consumes one
            # index per output partition) ----
            emb = sb.tile([128, F * D], f32, tag="emb")
            for f in range(F):
                nc.gpsimd.indirect_dma_start(
                    out=emb[:, f * D:(f + 1) * D],
                    out_offset=None,
                    in_=table_d[:, :],
                    in_offset=bass.IndirectOffsetOnAxis(
                        ap=idx_t[:, s * F + f:s * F + f + 1], axis=0
                    ),
                )

            # ---- numeric-field embedding (transposed): num_embT [64, 128] ----
            nrep = psm.tile([D, 128], f32, tag="small")
            nc.tensor.matmul(
                out=nrep[:], lhsT=ones_t[:],
                rhs=numf_t[:, s * BLK:(s + 1) * BLK],
                start=True, stop=True,
            )
            numembT = sb.tile([D, 128], f32, tag="numembT")
            nc.scalar.activation(
                out=numembT[:], in_=nrep[:], func=AF.Identity,
                bias=numb_c, scale=numw_c,
            )

            # ---- 2. transposes + evict; 3. squares ----
            # et layout: [128, 25*256] chunks [embT(128) | sq(128)]
            et = sb.tile([128, NCHUNK * 256], f32, tag="et")
            etv = et[:].rearrange("p (j c) -> p j c", c=256)
            for gi, (g0, gn) in enumerate(GROUPS):
                trp = pst.tile([128, 512], f32, tag="tr")
                for jj in range(gn):
                    j = g0 + jj
                    nc.tensor.matmul(
                        out=r(trp[:, jj * 128:(jj + 1) * 128]),
                        lhsT=r(emb[:, j * 128:(j + 1) * 128]),
                        rhs=r(ident_t[:]),
                        is_transpose=True, start=True, stop=True,
                    )
                src = trp[:, :gn * 128].rearrange("p (j c) -> p j c", c=128)
                nc.vector.tensor_copy(out=etv[:, g0:g0 + gn, 0:128], in_=src)
                if gi < 4:
                    # square on ACT straight from PSUM
                    nc.scalar.activation(
                        out=etv[:, g0:g0 + gn, 128:256], in_=src, func=AF.Square,
                    )
                else:
                    # square on DVE from SBUF (after evict)
                    nc.vector.tensor_tensor(
                        out=etv[:, g0:g0 + gn, 128:256],
                        in0=etv[:, g0:g0 + gn, 0:128],
                        in1=etv[:, g0:g0 + gn, 0:128],
                        op=AL.mult,
                    )

            # ---- 4. seg-matmuls: [sumT | sumsqT] accumulate in [64, 256] ----
            seg = pseg.tile([D, 256], f32, tag="seg")
            for j in range(NCHUNK):
                nc.tensor.matmul(
                    out=seg[:],
                    lhsT=r(segf_t[:]),
                    rhs=r(et[:, j * 256:(j + 1) * 256]),
                    start=(j == 0), stop=(j == NCHUNK - 1),
                    skip_group_check=True,
                )

            # ---- summedT (+aug ones row) ----
            saug = sb.tile([D + 1, 128], f32, tag="saug")
            nc.vector.tensor_tensor(
                out=saug[0:D, :], in0=seg[:, 0:128], in1=numembT[:], op=AL.add,
            )
            nc.vector.tensor_copy(out=saug[D:D + 1, :], in_=onesrow_t[:])

            # ---- 5. u-matmuls ----
            # bias: one N=512 matmul with rhs = summedT_aug repeated 4x
            saug_rep = (
                saug[:].rearrange("p (o n) -> p o n", o=1)
                .to_broadcast([D + 1, 4, 128])
            )
            r_buf = sb.tile([128, NCHUNK * 128], bf16, tag="rbuf")
            for gi, (g0, gn) in enumerate(GROUPS):
                up = psu.tile([128, 512], f32, tag="u")
                if gn == 4:
                    nc.tensor.matmul(
                        out=up[:], lhsT=r(waug_t[:]), rhs=r(saug_rep),
                        start=True, stop=False, skip_group_check=True,
                    )
                    for pp in range(2):  # support pairs N=256
                        rhs = et[:].rearrange("p (j c) -> p j c", c=256)[
                            :, g0 + 2 * pp:g0 + 2 * pp + 2, 0:128
                        ]
                        nc.tensor.matmul(
                            out=up[:, pp * 256:(pp + 1) * 256],
                            lhsT=r(gw2_t[:]), rhs=r(rhs),
                            start=False, stop=True, skip_group_check=True,
                        )
                else:
                    nc.tensor.matmul(
                        out=up[:, 0:128], lhsT=r(waug_t[:]), rhs=r(saug[:]),
                        start=True, stop=False, skip_group_check=True,
                    )
                    nc.tensor.matmul(
                        out=up[:, 0:128], lhsT=r(gw2_t[:]),
                        rhs=r(et[:, g0 * 256:g0 * 256 + 128]),
                        start=False, stop=True, skip_group_check=True,
                    )
                # ---- 6. relu-evict to bf16 ----
                nc.scalar.activation(
                    out=r_buf[:, g0 * 128:(g0 + gn) * 128],
                    in_=up[:, :gn * 128], func=AF.Relu,
                )

            # num field u + relu
            unum = psm.tile([D, 128], f32, tag="small")
            nc.tensor.matmul(
                out=unum[:], lhsT=r(waug_t[:, 0:D]), rhs=r(saug[:]),
                start=True, stop=False, skip_group_check=True,
            )
            nc.tensor.matmul(
                out=unum[:], lhsT=r(gw2_t[0:D, 0:D]), rhs=r(numembT[:]),
                start=False, stop=True, skip_group_check=True,
            )
            rnum = sb.tile([D, 128], bf16, tag="rnum")
            nc.scalar.activation(out=rnum[:], in_=unum[:], func=AF.Relu)

            # ---- racc: g_preT = sum_f relu(u_f) ----
            gpre = pracc.tile([D, 128], f32, tag="gpre")
            for j in range(NCHUNK):
                nc.tensor.matmul(
                    out=gpre[:], lhsT=segb_t[:], rhs=r_buf[:, j * 128:(j + 1) * 128],
                    start=(j == 0), stop=False, skip_group_check=True,
                )
            nc.tensor.matmul(
                out=gpre[:], lhsT=i64b_t[:], rhs=rnum[:],
                start=False, stop=True, skip_group_check=True,
            )
            gpreT = sb.tile([D, 128], f32, tag="gpreT")
            nc.scalar.copy(out=gpreT[:], in_=gpre[:])

            # ---- local branch: lT = summedT^2 - sumsqT ----
            lT = sb.tile([D, 128], f32, tag="lT")
            nc.vector.tensor_tensor(
                out=lT[:], in0=saug[0:D, :], in1=saug[0:D, :], op=AL.mult,
            )
            nc.vector.tensor_tensor(
                out=lT[:], in0=lT[:], in1=seg[:, 128:256], op=AL.subtract,
            )

            # ---- MLPs (transposed) ----
            h1p = psm.tile([D, 128], f32, tag="small")
            nc.tensor.matmul(out=h1p[:], lhsT=g0_t[:], rhs=gpreT[:],
                             start=True, stop=True)
            h1aug = sb.tile([D + 1, 128], f32, tag="h1aug")
            nc.scalar.activation(out=h1aug[0:D, :], in_=h1p[:], func=AF.Relu,
                                 bias=gb0_c)
            nc.vector.tensor_copy(out=h1aug[D:D + 1, :], in_=onesrow_t[:])

            l1p = psm.tile([D, 128], f32, tag="small")
            nc.tensor.matmul(out=l1p[:], lhsT=l0_t[:], rhs=lT[:],
                             start=True, stop=True)
            l1aug = sb.tile([D + 1, 128], f32, tag="l1aug")
            nc.scalar.activation(out=l1aug[0:D, :], in_=l1p[:], func=AF.Relu,
                                 bias=lb0_c)
            nc.vector.tensor_copy(out=l1aug[D:D + 1, :], in_=onesrow_t[:])

            outp = psm.tile([D, 128], f32, tag="small")
            nc.tensor.matmul(out=outp[:], lhsT=g1_t[:], rhs=h1aug[:],
                             start=True, stop=False, skip_group_check=True)
            nc.tensor.matmul(out=outp[:], lhsT=l1_t[:], rhs=l1aug[:],
                             start=False, stop=True, skip_group_check=True)
            outT = sb.tile([D, 128], f32, tag="outT")
            nc.scalar.copy(out=outT[:], in_=outp[:])

            # ---- transpose back to [128, 64] and store ----
            finp = psm.tile([128, D], f32, tag="small")
            nc.tensor.matmul(out=finp[:], lhsT=outT[:], rhs=i64f_t[:],
                             is_transpose=True, start=True, stop=True)
            orow = sb.tile([128, D], f32, tag="orow")
            nc.vector.tensor_copy(out=orow[:], in_=finp[:])
            nc.sync.dma_start(out_d[blk * BLK:(blk + 1) * BLK, :], orow[:])

        if rep_cm is not None:
            rep_cm.__exit__(None, None, None)

    return nc


def _make_consts(embed_table, num_W, num_b, ga_W, ga_b, gW, gb, lW, lb):
    """Host-side constant prep. Returns dict of name -> np.ndarray."""
    f = np.float32
    ga_W = ga_W.astype(f)
    ident128 = np.eye(128, dtype=f)
    i64 = np.eye(D, dtype=f)
    seg = np.vstack([i64, i64]).astype(f)           # [128, 64]
    waug = np.zeros((D + 1, 128), f)                # bias matmul lhsT
    waug[:D, :D] = ga_W / CD
    waug[:D, D:] = ga_W / CD
    waug[D, :D] = ga_b
    waug[D, D:] = ga_b
    gw2 = np.zeros((128, 128), f)                   # blockdiag support lhsT
    gw2[:D, :D] = ga_W * (PROBE / CD)
    gw2[D:, D:] = ga_W * (PROBE / CD)
    g0T = (gW[0].astype(f) / NF).T.copy()           # fold 1/51 mean
    g1aug = np.zeros((D + 1, D), f)
    g1aug[:D] = ALPHA * gW[1].astype(f).T
    g1aug[D] = ALPHA * gb[1].astype(f)
    l0T = (0.5 * lW[0].astype(f)).T.copy()          # fold FM 0.5
    l1aug = np.zeros((D + 1, D), f)
    l1aug[:D] = (1.0 - ALPHA) * lW[1].astype(f).T
    l1aug[D] = (1.0 - ALPHA) * lb[1].astype(f)
    cols = np.stack(
        [num_W[:, 0].astype(f), num_b.astype(f), gb[0].astype(f), lb[0].astype(f)],
        axis=1,
    ).copy()                                        # [64, 4]
    return {
        "table": np.ascontiguousarray(embed_table.astype(f)),
        "ident128": ident128,
        "seg_f": seg,
        "seg_b": seg,          # cast to bf16 at map build
        "i64_b": i64,          # cast to bf16 at map build
        "i64_f": i64,
        "waug": waug,
        "gw2": gw2,
        "g0T": g0T,
        "g1aug": g1aug,
        "l0T": l0T,
        "l1aug": l1aug,
        "cols": cols,
        "ones164": np.ones((1, D), f),
    }


def _get_nc():
    if "nc" not in _CACHE:
        print("[kernel] building bass module...", flush=True)
        nc = _build()
        print("[kernel] finalizing...", flush=True)
        nc.finalize()
        _CACHE["nc"] = nc
        print("[kernel] build done", flush=True)
    return _CACHE["nc"]


def _make_in_maps(inputs):
    """inputs: dict with the reference's setup_inputs() keys."""
    import ml_dtypes

    consts = _make_consts(
        inputs["embed_table"], inputs["num_W"], inputs["num_b"],
        inputs["ga_W"], inputs["ga_b"], inputs["gW"], inputs["gb"],
        inputs["lW"], inputs["lb"],
    )
    bf = ml_dtypes.bfloat16
    cmap = {
        k: (v.astype(bf) if k in ("seg_b", "i64_b") else v)
        for k, v in consts.items()
    }

    idx32 = np.ascontiguousarray(np.asarray(inputs["cat_indices"]).astype(np.int32))
    numf = np.ascontiguousarray(
        np.asarray(inputs["num_features"]).astype(np.float32))

    in_maps = []
    for c in range(NCORES):
        m = dict(cmap)
        m["cat_idx"] = np.ascontiguousarray(idx32[c * BS:(c + 1) * BS])
        m["numf"] = np.ascontiguousarray(numf[c * BS:(c + 1) * BS])
        in_maps.append(m)
    return in_maps


def kernel(cat_indices, num_features, embed_table, num_W, num_b,
           ga_W, ga_b, gW, gb, lW, lb):
    from concourse.bass_utils import run_bass_kernel_spmd

    nc = _get_nc()
    in_maps = _make_in_maps(dict(
        cat_indices=cat_indices, num_features=num_features,
        embed_table=embed_table, num_W=num_W, num_b=num_b,
        ga_W=ga_W, ga_b=ga_b, gW=gW, gb=gb, lW=lW, lb=lb,
    ))

    print("[kernel] launching spmd run...", flush=True)
    res = run_bass_kernel_spmd(nc, in_maps, list(range(NCORES)))
    print("[kernel] run complete", flush=True)
    outs = [res.results[c]["out"] for c in range(NCORES)]
    return np.concatenate(outs, axis=0).astype(np.float32)



# revision 16
# speedup vs baseline: 1.3998x; 1.0063x over previous
"""Trainium2 Bass kernel for nn_CatEmbedder (gnn_message_passing).

Takes FULL inputs, shards batch B=32768 across 8 NeuronCores (4096 each),
replicates the embedding table + weights, runs an SPMD Bass kernel, and
concatenates the per-core outputs.

Per-core pipeline (32 blocks x 128 samples):
  1. indirect-DMA gather: emb[p, f*64:(f+1)*64] = table[idx[p,f]]  ([128,3200])
  2. PE transposes of [128,128] chunks (2 fields each) -> PSUM -> SBUF (et)
  3. squares (ACT/DVE split) into interleaved sq chunks
  4. PE seg-matmuls: field-sum + field-sum-of-squares  -> [64,256] PSUM
  5. PE u-matmuls: u_f = (S + PROBE*support_f)/c + ga_b  (bias via aug row)
  6. ACT relu-evict (bf16) -> PE accumulates sum_f relu(u_f)
  7. transposed MLPs for global/local branches, combine, transpose back, store
"""

import os
import sys
import numpy as np

sys.path.insert(0, "/opt/trn_rl_repo")

# ---- problem constants (hardcoded per the contract) ----
B, F, D, NCT = 32768, 50, 64, 100000
PROBE, ALPHA = 39.0, 0.5
NF = F + 1              # 51 fields
CD = NF + PROBE         # 90.0
NCORES = 8
BS = B // NCORES        # 4096 samples per core
BLK = 128
SUPER = 8               # idx/numf superblock (blocks per DMA)
NBLK_FULL = BS // BLK   # 32

USE_F32R = False        # fast fp32 matmul mode (walrus needs fp32r-typed producers)

_CACHE = {}


def _build(nblk=NBLK_FULL, reps=1):
    import concourse.bass as bass
    import concourse.mybir as mybir
    import concourse.tile as tile
    from concourse import bacc
    from contextlib import ExitStack

    f32 = mybir.dt.float32
    f32r = mybir.dt.float32r
    bf16 = mybir.dt.bfloat16
    i32 = mybir.dt.int32
    AL = mybir.AluOpType
    AF = mybir.ActivationFunctionType

    def r(ap):  # fp32 -> fp32r view for fast matmuls
        return ap.bitcast(f32r) if USE_F32R else ap

    nc = bacc.Bacc(None)

    # ---- DRAM parameters (order matters only for debug; bound by name) ----
    idx_d = nc.declare_dram_parameter("cat_idx", [BS, F], i32, isOutput=False)
    numf_d = nc.declare_dram_parameter("numf", [BS], f32, isOutput=False)
    table_d = nc.declare_dram_parameter("table", [NCT, D], f32, isOutput=False)
    ident_d = nc.declare_dram_parameter("ident128", [128, 128], f32, isOutput=False)
    segf_d = nc.declare_dram_parameter("seg_f", [128, D], bf16, isOutput=False)
    segb_d = nc.declare_dram_parameter("seg_b", [128, D], bf16, isOutput=False)
    i64b_d = nc.declare_dram_parameter("i64_b", [D, D], bf16, isOutput=False)
    i64f_d = nc.declare_dram_parameter("i64_f", [D, D], f32, isOutput=False)
    waug_d = nc.declare_dram_parameter("waug", [D + 1, 128], bf16, isOutput=False)
    gw2_d = nc.declare_dram_parameter("gw2", [128, 128], bf16, isOutput=False)
    g0_d = nc.declare_dram_parameter("g0T", [D, D], f32, isOutput=False)
    g1_d = nc.declare_dram_parameter("g1aug", [D + 1, D], f32, isOutput=False)
    l0_d = nc.declare_dram_parameter("l0T", [D, D], f32, isOutput=False)
    l1_d = nc.declare_dram_parameter("l1aug", [D + 1, D], f32, isOutput=False)
    cols_d = nc.declare_dram_parameter("cols", [D, 4], f32, isOutput=False)
    ones_d = nc.declare_dram_parameter("ones164", [1, D], f32, isOutput=False)
    out_d = nc.declare_dram_parameter("out", [BS, D], f32, isOutput=True)

    GROUPS = [(0, 4), (4, 4), (8, 4), (12, 4), (16, 4), (20, 4), (24, 1)]
    NCHUNK = 25  # 25 chunks of 128 cols (2 fields each)

    with tile.TileContext(nc) as tc, ExitStack() as ctx:
        const = ctx.enter_context(tc.tile_pool(name="const", bufs=1))
        sb = ctx.enter_context(tc.tile_pool(name="sb", bufs=3))
        pst = ctx.enter_context(tc.tile_pool(name="pst", bufs=2, space="PSUM"))
        psu = ctx.enter_context(tc.tile_pool(name="psu", bufs=2, space="PSUM"))
        pseg = ctx.enter_context(tc.tile_pool(name="pseg", bufs=1, space="PSUM"))
        pracc = ctx.enter_context(tc.tile_pool(name="pracc", bufs=1, space="PSUM"))
        psm = ctx.enter_context(tc.tile_pool(name="psm", bufs=2, space="PSUM"))

        # ---- load constants once ----
        ident_t = const.tile([128, 128], f32)
        nc.sync.dma_start(ident_t[:], ident_d[:])
        segf_t = const.tile([128, D], bf16)
        nc.sync.dma_start(segf_t[:], segf_d[:])
        segb_t = const.tile([128, D], bf16)
        nc.sync.dma_start(segb_t[:], segb_d[:])
        i64b_t = const.tile([D, D], bf16)
        nc.sync.dma_start(i64b_t[:], i64b_d[:])
        i64f_t = const.tile([D, D], f32)
        nc.sync.dma_start(i64f_t[:], i64f_d[:])
        waug_t = const.tile([D + 1, 128], bf16)
        nc.sync.dma_start(waug_t[:], waug_d[:])
        gw2_t = const.tile([128, 128], bf16)
        nc.sync.dma_start(gw2_t[:], gw2_d[:])
        g0_t = const.tile([D, D], f32)
        nc.sync.dma_start(g0_t[:], g0_d[:])
        g1_t = const.tile([D + 1, D], f32)
        nc.sync.dma_start(g1_t[:], g1_d[:])
        l0_t = const.tile([D, D], f32)
        nc.sync.dma_start(l0_t[:], l0_d[:])
        l1_t = const.tile([D + 1, D], f32)
        nc.sync.dma_start(l1_t[:], l1_d[:])
        cols_t = const.tile([D, 4], f32)
        nc.sync.dma_start(cols_t[:], cols_d[:])
        ones_t = const.tile([1, D], f32)
        nc.sync.dma_start(ones_t[:], ones_d[:])
        onesrow_t = const.tile([1, 128], f32)
        nc.vector.memset(onesrow_t[:], 1.0)

        numw_c = cols_t[:, 0:1]
        numb_c = cols_t[:, 1:2]
        gb0_c = cols_t[:, 2:3]
        lb0_c = cols_t[:, 3:4]

        idx_view = idx_d[:].rearrange("(s k p) f -> s p k f", p=BLK, k=SUPER)

        idx_t = None
        numf_t = None
        rep_cm = tc.For_i(0, reps, 1) if reps > 1 else None
        if rep_cm is not None:
            rep_cm.__enter__()
        for blk in range(nblk):
            s = blk % SUPER
            if s == 0:
                si = blk // SUPER
                idx_t = sb.tile([128, SUPER * F], i32, tag="idx")
                nc.sync.dma_start(
                    idx_t[:].rearrange("p (k f) -> p k f", k=SUPER), idx_view[si]
                )
                numf_t = sb.tile([1, SUPER * BLK], f32, tag="numf")
                nc.sync.dma_start(
                    numf_t[:], numf_d[None, si * SUPER * BLK:(si + 1) * SUPER * BLK]
                )

            # ---- 1. gather (one indirect DMA per field: HW consumes one
            # index per output partition) ----
            emb = sb.tile([128, F * D], f32, tag="emb")
            for f in range(F):
                nc.gpsimd.indirect_dma_start(
                    out=emb[:, f * D:(f + 1) * D],
                    out_offset=None,
                    in_=table_d[:, :],
                    in_offset=bass.IndirectOffsetOnAxis(
                        ap=idx_t[:, s * F + f:s * F + f + 1], axis=0
                    ),
                )

            # ---- numeric-field embedding (transposed): num_embT [64, 128] ----
            nrep = psm.tile([D, 128], f32, tag="small")
            nc.tensor.matmul(
                out=nrep[:], lhsT=ones_t[:],
                rhs=numf_t[:, s * BLK:(s + 1) * BLK],
                start=True, stop=True,
            )
            numembT = sb.tile([D, 128], f32, tag="numembT")
            nc.scalar.activation(
                out=numembT[:], in_=nrep[:], func=AF.Identity,
                bias=numb_c, scale=numw_c,
            )

            # ---- 2. transposes + evict; 3. squares ----
            # et layout: [128, 25*256] chunks [embT(128) | sq(128)]
            et = sb.tile([128, NCHUNK * 256], bf16, tag="et")
            etv = et[:].rearrange("p (j c) -> p j c", c=256)
            for gi, (g0, gn) in enumerate(GROUPS):
                trp = pst.tile([128, 512], f32, tag="tr")
                for jj in range(gn):
                    j = g0 + jj
                    nc.tensor.matmul(
                        out=r(trp[:, jj * 128:(jj + 1) * 128]),
                        lhsT=r(emb[:, j * 128:(j + 1) * 128]),
                        rhs=r(ident_t[:]),
                        is_transpose=True, start=True, stop=True,
                    )
                src = trp[:, :gn * 128].rearrange("p (j c) -> p j c", c=128)
                nc.vector.tensor_copy(out=etv[:, g0:g0 + gn, 0:128], in_=src)
                if gi < 4:
                    # square on ACT straight from PSUM
                    nc.scalar.activation(
                        out=etv[:, g0:g0 + gn, 128:256], in_=src, func=AF.Square,
                    )
                else:
                    # square on DVE from SBUF (after evict)
                    nc.vector.tensor_tensor(
                        out=etv[:, g0:g0 + gn, 128:256],
                        in0=etv[:, g0:g0 + gn, 0:128],
                        in1=etv[:, g0:g0 + gn, 0:128],
                        op=AL.mult,
                    )

            # ---- 4. seg-matmuls: [sumT | sumsqT] accumulate in [64, 256] ----
            seg = pseg.tile([D, 256], f32, tag="seg")
            for j in range(NCHUNK):
                nc.tensor.matmul(
                    out=seg[:],
                    lhsT=r(segf_t[:]),
                    rhs=r(et[:, j * 256:(j + 1) * 256]),
                    start=(j == 0), stop=(j == NCHUNK - 1),
                    skip_group_check=True,
                )

            # ---- summedT (+aug ones row) ----
            saug = sb.tile([D + 1, 128], bf16, tag="saug")
            nc.vector.tensor_tensor(
                out=saug[0:D, :], in0=seg[:, 0:128], in1=numembT[:], op=AL.add,
            )
            nc.vector.tensor_copy(out=saug[D:D + 1, :], in_=onesrow_t[:])

            # ---- 5. u-matmuls ----
            # bias: one N=512 matmul with rhs = summedT_aug repeated 4x
            saug_rep = (
                saug[:].rearrange("p (o n) -> p o n", o=1)
                .to_broadcast([D + 1, 4, 128])
            )
            r_buf = sb.tile([128, NCHUNK * 128], bf16, tag="rbuf")
            for gi, (g0, gn) in enumerate(GROUPS):
                up = psu.tile([128, 512], f32, tag="u")
                if gn == 4:
                    nc.tensor.matmul(
                        out=up[:], lhsT=r(waug_t[:]), rhs=r(saug_rep),
                        start=True, stop=False, skip_group_check=True,
                    )
                    for pp in range(2):  # support pairs N=256
                        rhs = et[:].rearrange("p (j c) -> p j c", c=256)[
                            :, g0 + 2 * pp:g0 + 2 * pp + 2, 0:128
                        ]
                        nc.tensor.matmul(
                            out=up[:, pp * 256:(pp + 1) * 256],
                            lhsT=r(gw2_t[:]), rhs=r(rhs),
                            start=False, stop=True, skip_group_check=True,
                        )
                else:
                    nc.tensor.matmul(
                        out=up[:, 0:128], lhsT=r(waug_t[:]), rhs=r(saug[:]),
                        start=True, stop=False, skip_group_check=True,
                    )
                    nc.tensor.matmul(
                        out=up[:, 0:128], lhsT=r(gw2_t[:]),
                        rhs=r(et[:, g0 * 256:g0 * 256 + 128]),
                        start=False, stop=True, skip_group_check=True,
                    )
                # ---- 6. relu-evict to bf16 ----
                nc.scalar.activation(
                    out=r_buf[:, g0 * 128:(g0 + gn) * 128],
                    in_=up[:, :gn * 128], func=AF.Relu,
                )

            # num field u + relu
            unum = psm.tile([D, 128], f32, tag="small")
            nc.tensor.matmul(
                out=unum[:], lhsT=r(waug_t[:, 0:D]), rhs=r(saug[:]),
                start=True, stop=False, skip_group_check=True,
            )
            numembT16 = sb.tile([D, 128], bf16, tag="net16")
            nc.vector.tensor_copy(out=numembT16[:], in_=numembT[:])
            nc.tensor.matmul(
                out=unum[:], lhsT=r(gw2_t[0:D, 0:D]), rhs=r(numembT16[:]),
                start=False, stop=True, skip_group_check=True,
            )
            rnum = sb.tile([D, 128], bf16, tag="rnum")
            nc.scalar.activation(out=rnum[:], in_=unum[:], func=AF.Relu)

            # ---- racc: g_preT = sum_f relu(u_f) ----
            gpre = pracc.tile([D, 128], f32, tag="gpre")
            for j in range(NCHUNK):
                nc.tensor.matmul(
                    out=gpre[:], lhsT=segb_t[:], rhs=r_buf[:, j * 128:(j + 1) * 128],
                    start=(j == 0), stop=False, skip_group_check=True,
                )
            nc.tensor.matmul(
                out=gpre[:], lhsT=i64b_t[:], rhs=rnum[:],
                start=False, stop=True, skip_group_check=True,
            )
            gpreT = sb.tile([D, 128], f32, tag="gpreT")
            nc.scalar.copy(out=gpreT[:], in_=gpre[:])

            # ---- local branch: lT = summedT^2 - sumsqT ----
            lT = sb.tile([D, 128], f32, tag="lT")
            nc.vector.tensor_tensor(
                out=lT[:], in0=saug[0:D, :], in1=saug[0:D, :], op=AL.mult,
            )
            nc.vector.tensor_tensor(
                out=lT[:], in0=lT[:], in1=seg[:, 128:256], op=AL.subtract,
            )

            # ---- MLPs (transposed) ----
            h1p = psm.tile([D, 128], f32, tag="small")
            nc.tensor.matmul(out=h1p[:], lhsT=g0_t[:], rhs=gpreT[:],
                             start=True, stop=True)
            h1aug = sb.tile([D + 1, 128], f32, tag="h1aug")
            nc.scalar.activation(out=h1aug[0:D, :], in_=h1p[:], func=AF.Relu,
                                 bias=gb0_c)
            nc.vector.tensor_copy(out=h1aug[D:D + 1, :], in_=onesrow_t[:])

            l1p = psm.tile([D, 128], f32, tag="small")
            nc.tensor.matmul(out=l1p[:], lhsT=l0_t[:], rhs=lT[:],
                             start=True, stop=True)
            l1aug = sb.tile([D + 1, 128], f32, tag="l1aug")
            nc.scalar.activation(out=l1aug[0:D, :], in_=l1p[:], func=AF.Relu,
                                 bias=lb0_c)
            nc.vector.tensor_copy(out=l1aug[D:D + 1, :], in_=onesrow_t[:])

            outp = psm.tile([D, 128], f32, tag="small")
            nc.tensor.matmul(out=outp[:], lhsT=g1_t[:], rhs=h1aug[:],
                             start=True, stop=False, skip_group_check=True)
            nc.tensor.matmul(out=outp[:], lhsT=l1_t[:], rhs=l1aug[:],
                             start=False, stop=True, skip_group_check=True)
            outT = sb.tile([D, 128], f32, tag="outT")
            nc.scalar.copy(out=outT[:], in_=outp[:])

            # ---- transpose back to [128, 64] and store ----
            finp = psm.tile([128, D], f32, tag="small")
            nc.tensor.matmul(out=finp[:], lhsT=outT[:], rhs=i64f_t[:],
                             is_transpose=True, start=True, stop=True)
            orow = sb.tile([128, D], f32, tag="orow")
            nc.vector.tensor_copy(out=orow[:], in_=finp[:])
            nc.sync.dma_start(out_d[blk * BLK:(blk + 1) * BLK, :], orow[:])

        if rep_cm is not None:
            rep_cm.__exit__(None, None, None)

    return nc


def _make_consts(embed_table, num_W, num_b, ga_W, ga_b, gW, gb, lW, lb):
    """Host-side constant prep. Returns dict of name -> np.ndarray."""
    f = np.float32
    ga_W = ga_W.astype(f)
    ident128 = np.eye(128, dtype=f)
    i64 = np.eye(D, dtype=f)
    seg = np.vstack([i64, i64]).astype(f)           # [128, 64]
    waug = np.zeros((D + 1, 128), f)                # bias matmul lhsT
    waug[:D, :D] = ga_W / CD
    waug[:D, D:] = ga_W / CD
    waug[D, :D] = ga_b
    waug[D, D:] = ga_b
    gw2 = np.zeros((128, 128), f)                   # blockdiag support lhsT
    gw2[:D, :D] = ga_W * (PROBE / CD)
    gw2[D:, D:] = ga_W * (PROBE / CD)
    g0T = (gW[0].astype(f) / NF).T.copy()           # fold 1/51 mean
    g1aug = np.zeros((D + 1, D), f)
    g1aug[:D] = ALPHA * gW[1].astype(f).T
    g1aug[D] = ALPHA * gb[1].astype(f)
    l0T = (0.5 * lW[0].astype(f)).T.copy()          # fold FM 0.5
    l1aug = np.zeros((D + 1, D), f)
    l1aug[:D] = (1.0 - ALPHA) * lW[1].astype(f).T
    l1aug[D] = (1.0 - ALPHA) * lb[1].astype(f)
    cols = np.stack(
        [num_W[:, 0].astype(f), num_b.astype(f), gb[0].astype(f), lb[0].astype(f)],
        axis=1,
    ).copy()                                        # [64, 4]
    return {
        "table": np.ascontiguousarray(embed_table.astype(f)),
        "ident128": ident128,
        "seg_f": seg,
        "seg_b": seg,          # cast to bf16 at map build
        "i64_b": i64,          # cast to bf16 at map build
        "i64_f": i64,
        "waug": waug,
        "gw2": gw2,
        "g0T": g0T,
        "g1aug": g1aug,
        "l0T": l0T,
        "l1aug": l1aug,
        "cols": cols,
        "ones164": np.ones((1, D), f),
    }


def _get_nc():
    if "nc" not in _CACHE:
        print("[kernel] building bass module...", flush=True)
        nc = _build()
        print("[kernel] finalizing...", flush=True)
        nc.finalize()
        _CACHE["nc"] = nc
        print("[kernel] build done", flush=True)
    return _CACHE["nc"]


def _make_in_maps(inputs):
    """inputs: dict with the reference's setup_inputs() keys."""
    import ml_dtypes

    consts = _make_consts(
        inputs["embed_table"], inputs["num_W"], inputs["num_b"],
        inputs["ga_W"], inputs["ga_b"], inputs["gW"], inputs["gb"],
        inputs["lW"], inputs["lb"],
    )
    bf = ml_dtypes.bfloat16
    cmap = {
        k: (v.astype(bf) if k in ("seg_b", "i64_b", "seg_f", "waug", "gw2") else v)
        for k, v in consts.items()
    }

    idx32 = np.ascontiguousarray(np.asarray(inputs["cat_indices"]).astype(np.int32))
    numf = np.ascontiguousarray(
        np.asarray(inputs["num_features"]).astype(np.float32))

    in_maps = []
    for c in range(NCORES):
        m = dict(cmap)
        m["cat_idx"] = np.ascontiguousarray(idx32[c * BS:(c + 1) * BS])
        m["numf"] = np.ascontiguousarray(numf[c * BS:(c + 1) * BS])
        in_maps.append(m)
    return in_maps


def kernel(cat_indices, num_features, embed_table, num_W, num_b,
           ga_W, ga_b, gW, gb, lW, lb):
    from concourse.bass_utils import run_bass_kernel_spmd

    nc = _get_nc()
    in_maps = _make_in_maps(dict(
        cat_indices=cat_indices, num_features=num_features,
        embed_table=embed_table, num_W=num_W, num_b=num_b,
        ga_W=ga_W, ga_b=ga_b, gW=gW, gb=gb, lW=lW, lb=lb,
    ))

    print("[kernel] launching spmd run...", flush=True)
    res = run_bass_kernel_spmd(nc, in_maps, list(range(NCORES)))
    print("[kernel] run complete", flush=True)
    outs = [res.results[c]["out"] for c in range(NCORES)]
    return np.concatenate(outs, axis=0).astype(np.float32)

